# revision 65
# baseline (speedup 1.0000x reference)
"""NeRF volumetric alpha-compositing kernel for Trainium2 (Bass/Tile).

Full inputs:  rgbo [131072, 128, 4] f32, depth [131072, 128] f32.
Full output:  [131072, 3] f32.

Sharding: data-parallel over rays, 8 cores x 16384 rays.

Per-core algorithm (ray-per-partition layout; T ray-blocks of S=128 samples
on the free dim per superblock of BLOCK*T rays):
  delta[s]  = depth[s+1] - depth[s]; delta[S-1] = 1e9     (GPSIMD TT+memset)
  m[s]      = delta[s] * opacity[s]                       (GPSIMD TT)
  cs        = segmented inclusive cumsum of m             (DVE masked scan:
              state = mask*state + m, mask=0 at block starts)
  th_c      = tanh(0.5 * rgb_c), bf16                     (ScalarE)
  teh[0]=0.5; teh[i+1] = 0.5*exp(-cs[i])                  (ScalarE Exp,
              scale=-1, bias=ln(1/2); exp+tanh share one act table set)
  w[i]      = teh[i] - teh[i+1], bf16 [= T_i*alpha_i / 2] (DVE TT)
  out[t,c]  = sum_s th_c*w  +  (teh[t,0] - teh[t,S])      (DVE bf16 TT mult +
              per-block tensor_reduce(axis=X); the sum_s w term telescopes
              to teh[t,0]-teh[t,S] since w is the difference seq of teh)

sigmoid(x)=(1+tanh(x/2))/2 keeps all activations in the single
exp_and_others act-table set (exp+tanh), so the 1.3us LoadActFuncSet never
re-fires inside the loop.  bf16 on th/w halves DVE port traffic for the
multiply (2x_1p packed mode); the telescoped correction keeps the +1 term
exact in fp32 (teh stays fp32).  Input DMAs are issued on separate queues
from the output DMA so a blocked store never stalls the input stream; the
w/reduce stage of superblock N is emitted after the front of superblock N+2
(software pipelining, skew=2) so in-order engine queues never wait on a
same-iteration cross-engine round trip.
The last-sample FAR_DELTA=1e9 is exact: teh[S]=0.5*exp(-cs[S-1]) underflows
to 0 whenever opacity[S-1] > ~1e-7, else matches the reference expression.
HW-measured (differential hardware-loop timing): 188 us/core vs 253 us for
the previous kernel; DMA-only floor of this access pattern is ~128 us.
"""

import math
from contextlib import ExitStack

import numpy as np

import concourse.bass as bass
import concourse.tile as tile
from concourse import bacc, mybir
from concourse.bass_utils import run_bass_kernel_spmd

N_RAYS = 131072
S = 128
N_CORES = 8
NC_RAYS = N_RAYS // N_CORES  # 16384 rays per core
BLOCK = 128                  # rays per partition-block
F32 = mybir.dt.float32
BF16 = mybir.dt.bfloat16
LN_HALF = math.log(0.5)
Alu = mybir.AluOpType


def build_nerf_bass(
    n_rays: int = NC_RAYS,
    t_blocks: int = 8,
    repeat: int = 1,
    loop_iters: int = 0,
    bufs: int = 3,
    dma_only: bool = False,
    eng_delta: str = "gpsimd",
    eng_m: str = "gpsimd",
    eng_w: str = "vector",
    reduce_mode: str = "treduce",
    gpsimd_stt_ch: int = 0,
    rgbo_dma_split: int = 1,
    out_dma_engine: str = "scalar",
    depth_dma_engine: str = "sync",
    prefetch: int = 0,
    skew: int = 2,
    g_bufs: int | None = None,
    tail_plan: tuple = (4, 2, 1, 1),
    tail_low_lat: bool = True,
    bf16_gw: bool = True,
    skip_reduce: bool = False,
    pmajor: bool = False,
    depth_pair: int = 2,
    te_act: bool = False,
    small_eng: str = "vector",
    scan_mode: str = "masked",
) -> bass.Bass:
    """Build the per-core Bass program for n_rays rays."""
    T = t_blocks
    SUPER = BLOCK * T
    assert n_rays % SUPER == 0
    n_super = n_rays // SUPER
    F = S * T                 # free size of a [BLOCK, F] plane
    U = S + 4                 # padded per-block stride for the teh table

    nc = bacc.Bacc("TRN2", target_bir_lowering=False, debug=False)
    rgbo_h = nc.declare_dram_parameter("rgbo", [n_rays, S, 4], F32, isOutput=False)
    depth_h = nc.declare_dram_parameter("depth", [n_rays, S], F32, isOutput=False)
    out_h = nc.declare_dram_parameter("out", [n_rays, 3], F32, isOutput=True)

    rgbo_ap = rgbo_h.ap()
    depth_ap = depth_h.ap()
    out_ap = out_h.ap()

    with ExitStack() as ctx:
        tc = ctx.enter_context(tile.TileContext(nc))
        gb = g_bufs if g_bufs is not None else bufs + skew - (1 if pmajor else 0)
        d_bufs = max(2, bufs - 1) if (pmajor and depth_pair > 1) else bufs
        p_const = ctx.enter_context(tc.tile_pool(name="const", bufs=1))
        p_rgbo = ctx.enter_context(tc.tile_pool(name="rgbo", bufs=bufs))
        p_depth = ctx.enter_context(tc.tile_pool(name="depth", bufs=d_bufs))
        p_g = ctx.enter_context(tc.tile_pool(name="g", bufs=gb))
        p_mid = ctx.enter_context(tc.tile_pool(name="mid", bufs=bufs))
        p_te = ctx.enter_context(tc.tile_pool(name="te", bufs=gb))
        p_scr = ctx.enter_context(tc.tile_pool(name="scr", bufs=4))
        p_out = ctx.enter_context(tc.tile_pool(name="outp", bufs=bufs))

        # mask: 0.0 at each ray-block start (s==0), 1.0 elsewhere; the scan
        # state = mask*state + data resets the running sum per ray.
        mask_t = p_const.tile([BLOCK, F], F32, tag="mask")
        nc.vector.memset(mask_t[:], 1.0)
        nc.vector.memset(
            mask_t.rearrange("p (t s) -> p t s", t=T)[:, :, 0:1], 0.0
        )
        bias_t = p_const.tile([BLOCK, 1], F32, tag="bias")
        nc.vector.memset(bias_t[:], LN_HALF)
        half_t = p_const.tile([BLOCK, 1], F32, tag="half")
        nc.vector.memset(half_t[:], 0.5)

        ns = rgbo_dma_split

        # p-major: partition p owns rays [p*R, (p+1)*R); chunk k covers
        # per-partition rays [k*T, (k+1)*T). Output accumulates in one
        # persistent SBUF tile, stored with a single DMA at the end.
        R = n_rays // BLOCK
        if pmajor:
            rgbo_pm = rgbo_ap.rearrange("(p r) s c -> p r (s c)", p=BLOCK)
            depth_pm = depth_ap.rearrange("(p r) s -> p r s", p=BLOCK)
            out_pm_t = p_out.tile(
                [BLOCK, 3 * R], F32, tag="outall", bufs=1
            )
        depth_state = {"left": 0}

        def load_superblock(r0, Te):
            Fe = S * Te
            if pmajor:
                if depth_state["left"] >= Te:
                    depth_t = depth_state["dt"]
                    dco = depth_state["col"]
                    depth_state["col"] += Fe
                    depth_state["left"] -= Te
                else:
                    npair = min(depth_pair, (R - r0) // T) if Te == T else 1
                    depth_t = p_depth.tile(
                        [BLOCK, depth_pair * F], F32, tag="depth"
                    )
                    getattr(nc, depth_dma_engine).dma_start(
                        out=depth_t[:, 0 : npair * Te * S],
                        in_=depth_pm[:, r0 : r0 + npair * Te].rearrange(
                            "p r s -> p (r s)"
                        ),
                    )
                    dco = 0
                    depth_state.update(
                        dt=depth_t, col=Fe, left=npair * Te - Te
                    )
                rgbo_t = p_rgbo.tile([BLOCK, 4 * F], F32, tag="rgbo")
                nc.sync.dma_start(
                    out=rgbo_t[:, 0 : 4 * Fe],
                    in_=rgbo_pm[:, r0 : r0 + Te].rearrange("p r f -> p (r f)"),
                )
                return rgbo_t, (depth_t, dco)
            depth_t = p_depth.tile([BLOCK, F], F32, tag="depth")
            getattr(nc, depth_dma_engine).dma_start(
                out=depth_t[:, 0:Fe],
                in_=depth_ap[r0 : r0 + BLOCK * Te].rearrange(
                    "(p t) s -> p (t s)", p=BLOCK
                ),
            )
            rgbo_t = p_rgbo.tile([BLOCK, 4 * F], F32, tag="rgbo")
            rgbo_src = rgbo_ap[r0 : r0 + BLOCK * Te].rearrange(
                "(p t) s c -> p t (s c)", p=BLOCK
            )
            rgbo_dst = rgbo_t[:, 0 : 4 * Fe].rearrange("p (t f) -> p t f", t=Te)
            nse = min(ns, Te)
            for i in range(nse):
                lo, hi = i * Te // nse, (i + 1) * Te // nse
                nc.sync.dma_start(out=rgbo_dst[:, lo:hi], in_=rgbo_src[:, lo:hi])
            return rgbo_t, (depth_t, 0)

        def compute_superblock(r0, Te, rgbo_t, depth_pack, low_lat=False):
            Fe = S * Te
            depth_t, dco = depth_pack
            e_delta_n = "vector" if low_lat else eng_delta
            e_m_n = "vector" if low_lat else eng_m
            rgbo4 = rgbo_t[:, 0 : 4 * Fe].rearrange(
                "p (t s c) -> p t s c", t=Te, s=S, c=4
            )
            depth3 = depth_t[:, dco : dco + Fe].rearrange(
                "p (t s) -> p t s", t=Te
            )

            if dma_only:
                if pmajor:
                    nc.vector.scalar_tensor_tensor(
                        out=out_pm_t[:, 3 * r0 : 3 * r0 + 1],
                        in0=rgbo_t[:, 0:1], scalar=0.0,
                        in1=depth_t[:, 0:1], op0=Alu.mult, op1=Alu.add,
                    )
                    return
                out_t = p_out.tile([BLOCK, 3 * T], F32, tag="out")
                nc.vector.scalar_tensor_tensor(
                    out=out_t[:, 0:1], in0=rgbo_t[:, 0:1], scalar=0.0,
                    in1=depth_t[:, 0:1], op0=Alu.mult, op1=Alu.add,
                )
                nc.vector.memset(out_t[:, 1 : 3 * Te], 0.0)
                nc.sync.dma_start(
                    out=out_ap[r0 : r0 + BLOCK * Te].rearrange(
                        "(p t) c -> p (t c)", p=BLOCK
                    ),
                    in_=out_t[:, 0 : 3 * Te],
                )
                return

            # th_c = tanh(rgb_c / 2); sigmoid(x) = (1 + tanh(x/2)) / 2
            gw_dt = BF16 if bf16_gw else F32
            if reduce_mode == "packed":
                # one [p, t, c, s] tile so the multiply+reduce fuse channels
                g_all = p_g.tile([BLOCK, 3 * F], gw_dt, tag="gall")
                g4 = g_all[:, 0 : 3 * Fe].rearrange(
                    "p (t c s) -> p t c s", t=Te, c=3
                )
                for c in range(3):
                    nc.scalar.activation(
                        g4[:, :, c],
                        rgbo4[:, :, :, c],
                        mybir.ActivationFunctionType.Tanh,
                        scale=0.5,
                    )
                th = g_all
            else:
                th = []
                for c in range(3):
                    g_c = p_g.tile([BLOCK, F], gw_dt, tag=f"g{c}")
                    g3 = g_c[:, 0:Fe].rearrange("p (t s) -> p t s", t=Te)
                    nse = min(ns, Te)
                    for i in range(nse):
                        lo, hi = i * Te // nse, (i + 1) * Te // nse
                        nc.scalar.activation(
                            g3[:, lo:hi],
                            rgbo4[:, lo:hi, :, c],
                            mybir.ActivationFunctionType.Tanh,
                            scale=0.5,
                        )
                    th.append(g_c)

            # delta[s] = depth[s+1]-depth[s] (s<S-1); delta[S-1] = FAR 1e9
            delta_t = p_mid.tile([BLOCK, F], F32, tag="delta")
            delta3 = delta_t[:, 0:Fe].rearrange("p (t s) -> p t s", t=Te)
            e_delta = getattr(nc, e_delta_n)
            e_delta.tensor_sub(
                delta3[:, :, 0 : S - 1],
                depth3[:, :, 1:S],
                depth3[:, :, 0 : S - 1],
            )
            e_delta.memset(delta3[:, :, S - 1 : S], 1.0e9)

            # m = delta * opacity
            m_t = p_mid.tile([BLOCK, F], F32, tag="m")
            getattr(nc, e_m_n).tensor_mul(
                m_t[:, 0:Fe].rearrange("p (t s) -> p t s", t=Te),
                delta3,
                rgbo4[:, :, :, 3],
            )

            # cs = per-ray inclusive cumsum of m (masked segmented scan)
            cs_t = p_mid.tile([BLOCK, F], F32, tag="cs")
            if scan_mode == "masked":
                nc.vector.tensor_tensor_scan(
                    cs_t[:, 0:Fe], mask_t[:, 0:Fe], m_t[:, 0:Fe],
                    0.0, Alu.mult, Alu.add,
                )
            else:
                for t in range(Te):
                    nc.vector.tensor_tensor_scan(
                        cs_t[:, t * S : (t + 1) * S],
                        m_t[:, t * S : (t + 1) * S],
                        m_t[:, t * S : (t + 1) * S],
                        0.0, Alu.add, Alu.bypass,
                    )

            # teh[0] = 0.5; teh[i+1] = 0.5*exp(-cs[i])
            te_t = p_te.tile([BLOCK, U * T], F32, tag="te")
            te3 = te_t[:, 0 : U * Te].rearrange("p (t u) -> p t u", t=Te)
            if te_act:
                # Identity(0*x + 0.5) on the Activation engine
                nc.scalar.activation(
                    te3[:, :, 0:1],
                    mask_t[:, 0:Te],
                    mybir.ActivationFunctionType.Identity,
                    scale=0.0,
                    bias=half_t[:],
                )
            else:
                nc.vector.memset(te3[:, :, 0:1], 0.5)
            nc.scalar.activation(
                te3[:, :, 1 : S + 1],
                cs_t[:, 0:Fe].rearrange("p (t s) -> p t s", t=Te),
                mybir.ActivationFunctionType.Exp,
                scale=-1.0,
                bias=bias_t[:],
            )
            return th, te_t

        def tail_superblock(r0, Te, th, te_t, low_lat=False):
            Fe = S * Te
            gw_dt = BF16 if bf16_gw else F32
            te3 = te_t[:, 0 : U * Te].rearrange("p (t u) -> p t u", t=Te)
            # w[i] = teh[i] - teh[i+1]  (= 0.5 * T_i * alpha_i)
            w_t = p_mid.tile([BLOCK, F], gw_dt, tag="w")
            w3 = w_t[:, 0:Fe].rearrange("p (t s) -> p t s", t=Te)
            getattr(nc, "vector" if low_lat else eng_w).tensor_sub(
                w3,
                te3[:, :, 0:S],
                te3[:, :, 1 : S + 1],
            )

            if pmajor:
                out_t = out_pm_t
                out3 = out_pm_t[:, 3 * r0 : 3 * (r0 + Te)].rearrange(
                    "p (t c) -> p t c", c=3
                )
            else:
                out_t = p_out.tile([BLOCK, 3 * T], F32, tag="out")
                out3 = out_t[:, 0 : 3 * Te].rearrange("p (t c) -> p t c", c=3)
            if skip_reduce:
                nc.vector.memset(out3, 0.0)
            elif reduce_mode == "stt":
                # out[t,c] = sum_s (th_c+1)*w : one fused stt+accum per (t,c)
                for t in range(Te):
                    for c in range(3):
                        eng = nc.vector if c < 3 - gpsimd_stt_ch else nc.gpsimd
                        tag = "scr" if c < 3 - gpsimd_stt_ch else "scrg"
                        scr = p_scr.tile([BLOCK, S], gw_dt, tag=tag)
                        eng.scalar_tensor_tensor(
                            out=scr[:],
                            in0=th[c][:, 0:Fe].rearrange(
                                "p (t s) -> p t s", t=Te
                            )[:, t],
                            scalar=1.0,
                            in1=w3[:, t],
                            op0=Alu.add,
                            op1=Alu.mult,
                            accum_out=out3[:, t, c : c + 1],
                        )
            elif reduce_mode == "packed":
                # out[t,c] = sum_s th*w + (teh[t,0]-teh[t,S]): one broadcast
                # multiply + one segmented reduce covering all 3 channels.
                e_small = getattr(nc, small_eng)
                corr = p_scr.tile([BLOCK, T], F32, tag="corr")
                corr3 = corr[:, 0:Te].rearrange("p (t o) -> p t o", o=1)
                e_small.tensor_sub(
                    corr3, te3[:, :, 0:1], te3[:, :, S : S + 1]
                )
                g4 = th[:, 0 : 3 * Fe].rearrange(
                    "p (t c s) -> p t c s", t=Te, c=3
                )
                wb = w_t[:, 0:Fe].rearrange(
                    "p (t o s) -> p t o s", t=Te, o=1
                ).to_broadcast([BLOCK, Te, 3, S])
                wgt = p_mid.tile([BLOCK, 3 * F], gw_dt, tag="wgall")
                wgt4 = wgt[:, 0 : 3 * Fe].rearrange(
                    "p (t c s) -> p t c s", t=Te, c=3
                )
                nc.vector.tensor_mul(wgt4, g4, wb)
                red = p_scr.tile([BLOCK, 3 * T], F32, tag="red")
                red3 = red[:, 0 : 3 * Te].rearrange(
                    "p (t c o) -> p t c o", t=Te, c=3, o=1
                )
                nc.vector.tensor_reduce(
                    red3, wgt4, mybir.AxisListType.X, Alu.add
                )
                corr_b = corr[:, 0:Te].rearrange(
                    "p (t o) -> p t o", o=1
                ).to_broadcast([BLOCK, Te, 3])
                e_small.tensor_add(
                    out3, red[:, 0 : 3 * Te].rearrange(
                        "p (t c) -> p t c", c=3
                    ), corr_b,
                )
            elif reduce_mode == "treduce":
                # out[t,c] = sum_s th*w + (teh[t,0] - teh[t,S]);
                # sum_s w telescopes to teh[t,0]-teh[t,S] since w is the
                # difference sequence of teh.
                e_small = getattr(nc, small_eng)
                corr = p_scr.tile([BLOCK, T], F32, tag="corr")
                corr3 = corr[:, 0:Te].rearrange("p (t o) -> p t o", o=1)
                e_small.tensor_sub(
                    corr3, te3[:, :, 0:1], te3[:, :, S : S + 1]
                )
                for c in range(3):
                    wgt = p_mid.tile([BLOCK, F], gw_dt, tag=f"wg{c}")
                    wgt3 = wgt[:, 0:Fe].rearrange("p (t s) -> p t s", t=Te)
                    nc.vector.tensor_mul(
                        wgt3,
                        th[c][:, 0:Fe].rearrange("p (t s) -> p t s", t=Te),
                        w3,
                    )
                    red = p_scr.tile([BLOCK, T], F32, tag=f"red{c}")
                    red3 = red[:, 0:Te].rearrange("p (t o) -> p t o", o=1)
                    nc.vector.tensor_reduce(
                        red3, wgt3, mybir.AxisListType.X, Alu.add
                    )
                    e_small.tensor_add(out3[:, :, c : c + 1], red3, corr3)
            else:
                for c in range(3):
                    # wg = (th + 1) * w  (= T_i * alpha_i * sigmoid(rgb_c))
                    wg = p_mid.tile([BLOCK, F], F32, tag=f"wg{c}")
                    nc.vector.scalar_tensor_tensor(
                        out=wg[:, 0:Fe],
                        in0=th[c][:, 0:Fe],
                        scalar=1.0,
                        in1=w_t[:, 0:Fe],
                        op0=Alu.add,
                        op1=Alu.mult,
                    )
                    # segmented running sum; per-ray total lands at s = S-1
                    wgs = p_mid.tile([BLOCK, F], F32, tag=f"wgs{c}")
                    nc.vector.tensor_tensor_scan(
                        wgs[:, 0:Fe], mask_t[:, 0:Fe], wg[:, 0:Fe],
                        0.0, Alu.mult, Alu.add,
                    )
                    src = wgs[:, 0:Fe].rearrange("p (t s) -> p t s", t=Te)[
                        :, :, S - 1 : S
                    ]
                    nc.gpsimd.tensor_scalar_mul(out3[:, :, c : c + 1], src, 1.0)

            if not pmajor:
                getattr(nc, out_dma_engine).dma_start(
                    out=out_ap[r0 : r0 + BLOCK * Te].rearrange(
                        "(p t) c -> p (t c)", p=BLOCK
                    ),
                    in_=out_t[:, 0 : 3 * Te],
                )

        def emit_all():
            depth_state.clear()
            depth_state["left"] = 0
            sched = []
            n_chunks = R // T if pmajor else n_super
            for sb in range(n_chunks * repeat):
                r0 = (sb % n_chunks) * (T if pmajor else SUPER)
                last = sb == n_chunks * repeat - 1
                if last and tail_plan:
                    off = 0
                    for tp in tail_plan:
                        sched.append(
                            (r0 + off * (1 if pmajor else BLOCK), tp,
                             tail_low_lat)
                        )
                        off += tp
                    assert off == T
                else:
                    sched.append((r0, T, False))
            pend = []   # loaded, front not yet emitted
            tails = []  # front emitted, tail pending
            for r0, Te, ll in sched:
                pend.append(((r0, Te, ll), load_superblock(r0, Te)))
                if len(pend) > prefetch:
                    (r, t_, ll_), tl = pend.pop(0)
                    tails.append(
                        ((r, t_, ll_), compute_superblock(r, t_, *tl, low_lat=ll_))
                    )
                if len(tails) > skew:
                    (r, t_, ll_), tl = tails.pop(0)
                    if tl is not None:
                        tail_superblock(r, t_, *tl, low_lat=ll_)
            for (r, t_, ll_), tl in pend:
                tails.append(
                    ((r, t_, ll_), compute_superblock(r, t_, *tl, low_lat=ll_))
                )
            for (r, t_, ll_), tl in tails:
                if tl is not None:
                    tail_superblock(r, t_, *tl, low_lat=ll_)
            if pmajor:
                getattr(nc, out_dma_engine).dma_start(
                    out=out_ap.rearrange("(p r) c -> p (r c)", p=BLOCK),
                    in_=out_pm_t[:],
                )

        if loop_iters:
            with tc.For_i(0, loop_iters, 1) as _i:
                emit_all()
        else:
            emit_all()
    nc.compile()
    return nc


_NC_CACHE: dict = {}


def _get_nc():
    if "nc" not in _NC_CACHE:
        _NC_CACHE["nc"] = build_nerf_bass()
    return _NC_CACHE["nc"]


def kernel(rgbo: np.ndarray, depth: np.ndarray, **run_kwargs) -> np.ndarray:
    rgbo = np.ascontiguousarray(rgbo, dtype=np.float32)
    depth = np.ascontiguousarray(depth, dtype=np.float32)
    assert rgbo.shape == (N_RAYS, S, 4) and depth.shape == (N_RAYS, S)

    nc = _get_nc()
    in_maps = []
    for i in range(N_CORES):
        sl = slice(i * NC_RAYS, (i + 1) * NC_RAYS)
        in_maps.append({"rgbo": rgbo[sl], "depth": depth[sl]})
    res = run_bass_kernel_spmd(nc, in_maps, core_ids=list(range(N_CORES)), **run_kwargs)
    out = np.concatenate([r["out"] for r in res.results], axis=0)
    if run_kwargs:
        kernel.last_results = res  # stash for profiling harnesses
    return out
